# revision 17
# baseline (speedup 1.0000x reference)
"""Multi-head causal self-attention (B=4, T=1024, d_model=2048, 16 heads of 128)
for 8 Trainium2 NeuronCores.

Sharding: hybrid data x tensor parallel. Core c handles batch b = c//2 and
head group g = c%2 (8 heads per core). Each core computes q/k/v projections
for its 8 heads, causal flash-style attention, and the out-projection rows
for those heads, producing a partial [1024, 2048] output for its batch.
The host sums the two partials per batch and adds the output bias.

Precision: q/k projections run in fp8(e4m3) with DoubleRow matmuls (2 k-tiles
of 128 contracted per pass -> 2x PE throughput); softmax makes the resulting
~5% q/k noise nearly invisible in the output (rel err ~1e-2 vs the 2e-2 gate)
because attention-weight wiggle only re-mixes exact fp16 v rows. The v path,
scores, AV, denominator and out-projection stay fp16 (v and out-proj errors
pass straight through to the output, so fp8 there fails the gate). Weights
are pre-scaled by 64 before fp8 quantization (w ~ +-0.022 would be subnormal
in e4m3); the 1/64 and the 1/sqrt(dh) score scale are folded into the fused
scale+bias (scalar_tensor_tensor) that moves q/k from PSUM to SBUF.

All on-device layouts are feature-major so no transposes are needed anywhere:
  - x is shipped per batch as xt [2048, 1024] twice: fp16 (v path) and fp8
    in DoubleRow pair layout (q/k path)
  - q, k are produced feature-major [dh, T] per head; v token-major [T, dh]
  - scores are computed transposed: S^T[kv, q] = k_fm.T @ q_fm (lhsT=k, rhs=q)
  - softmax denominator via ones[128,128] matmul (partition reduction on PE),
    which also broadcasts the per-q sum to all 128 partitions
  - attention output accumulates as out^T[dh, q] = v_tm.T @ exp(S^T)
  - out^T is exactly the lhsT the out-projection needs

Heads are processed in two blocks of 4 so projection weights and q/k/v
activations fit in SBUF alongside the resident x^T and w_out. Within a
block, attention is computed for two heads interleaved so PE matmuls hide
the ACT exp latency. Inputs are DMA'd in per-k-chunk tiles so the first
projection matmuls start ~1us in instead of waiting for monolithic loads.
The output partial is written fp16 (halves the tail DMA); the host sums the
two partials per batch in fp32.
"""

import numpy as np
import ml_dtypes

B, T, C = 4, 1024, 2048
H = 16          # total heads
HL = 8          # heads per core (local)
HB = 4          # heads per block
DH = 128        # head dim
KC = C // 128   # contraction chunks (16)
KP = KC // 2    # DoubleRow chunk pairs (8)
P = 128
NCORES = 8
SW = 64.0       # fp8 weight pre-scale

_cache = {}


def _build():
    import concourse.bacc as bacc
    import concourse.mybir as mybir
    import concourse.tile as tile

    F32 = mybir.dt.float32
    F16 = mybir.dt.float16
    F8 = mybir.dt.float8e4
    AF = mybir.ActivationFunctionType
    ALU = mybir.AluOpType
    DR = mybir.MatmulPerfMode.DoubleRow

    rs = float(1.0 / np.sqrt(DH))

    nc = bacc.Bacc("TRN2", target_bir_lowering=False, debug=False)

    BW = HB * DH  # head-block feature width (512)

    # all inputs ship in partition-major pre-tiled layouts so every DMA is a
    # contiguous run on both the DRAM and SBUF side (max-size descriptors)
    xt_d = nc.dram_tensor("xt", (P, KC * T), F16, kind="ExternalInput")
    xt8_d = nc.dram_tensor("xt8", (P, KP * 2 * T), F8, kind="ExternalInput")
    wq_d = nc.dram_tensor("wq", (P, 2 * KP * 2 * BW), F8, kind="ExternalInput")
    wk_d = nc.dram_tensor("wk", (P, 2 * KP * 2 * BW), F8, kind="ExternalInput")
    wv_d = nc.dram_tensor("wv", (P, 2 * KC * BW), F16, kind="ExternalInput")
    wo_d = nc.dram_tensor("wo", (HL * DH, C), F16, kind="ExternalInput")
    bq_d = nc.dram_tensor("bq", (P, HL), F32, kind="ExternalInput")
    bk_d = nc.dram_tensor("bk", (P, HL), F32, kind="ExternalInput")
    bvb_d = nc.dram_tensor("bvb", (P, HL * DH), F32, kind="ExternalInput")
    mask_d = nc.dram_tensor("mask", (P, P), F32, kind="ExternalInput")
    part_d = nc.dram_tensor("part", (T, C), F16, kind="ExternalOutput")

    xt_v = xt_d.rearrange("p (o t) -> p o t", o=KC)
    xt8_v = xt8_d.rearrange("p (o two t) -> p o two t", o=KP, two=2)
    wq_v = wq_d.rearrange("p (b o two m) -> p b o two m", b=2, o=KP, two=2)
    wk_v = wk_d.rearrange("p (b o two m) -> p b o two m", b=2, o=KP, two=2)
    wv_v = wv_d.rearrange("p (b o m) -> p b o m", b=2, o=KC)

    with tile.TileContext(nc) as tc:
        with (
            tc.tile_pool(name="res", bufs=1) as res,
            tc.tile_pool(name="wblk", bufs=1) as wblk,
            tc.tile_pool(name="qkv", bufs=2) as qkv,
            tc.tile_pool(name="wp", bufs=3) as wp,
            tc.tile_pool(name="ps", bufs=3, space="PSUM") as ps,
        ):
            bq_sb = res.tile([P, HL], F32, tag="bq")
            bk_sb = res.tile([P, HL], F32, tag="bk")
            bvb_sb = res.tile([P, HL * DH], F32, tag="bvb")
            mask_sb = res.tile([P, P], F32, tag="mask")

            ones_sb = res.tile([P, P], F16, tag="ones")
            nc.vector.memset(ones_sb[:], 1.0)

            # Warm the PE (HAM un-throttles after ~3.4us of activity) while the
            # input DMAs stream in; these matmuls depend only on the memset.
            # The DMA path has a ~8us prolog before the first bytes land, so
            # size the warmup to roughly cover it at the ramp-up clock.
            warm = ps.tile([P, P], F32, tag="mm")
            for _ in range(80):
                nc.tensor.matmul(warm[:], ones_sb[:], ones_sb[:], start=True, stop=True)

            # x^T fp16 per-k-chunk (v path) and fp8 per-pair (q/k path)
            xts = []
            for kc in range(KC):
                xts.append(res.tile([P, T], F16, tag=f"xt{kc}", name=f"xt{kc}"))
            xt8s = []
            for o in range(KP):
                xt8s.append(res.tile([P, 2, T], F8, tag=f"xt8_{o}", name=f"xt8_{o}"))
            wts = {"wv": [None] * KC}
            w8s = {"wq": [None] * KP, "wk": [None] * KP}

            def dma_block_weights(blk):
                def load_w8(wname, wv_, o):
                    wt = wblk.tile(
                        [P, 2, BW], F8, tag=f"{wname}{o}", name=f"{wname}{o}_{blk}"
                    )
                    nc.sync.dma_start(wt[:], wv_[:, blk, o, :, :])
                    w8s[wname][o] = wt

                def load_wv(o):
                    wt = wblk.tile([P, BW], F16, tag=f"wv{o}", name=f"wv{o}_{blk}")
                    nc.sync.dma_start(wt[:], wv_v[:, blk, o, :])
                    wts["wv"][o] = wt

                if blk == 0:
                    # arrival order matches first consumption: the h=0 q-proj
                    # chain needs (xt8[o], wq[o]) pairs in o order
                    for o in range(KP):
                        nc.sync.dma_start(xt8s[o][:], xt8_v[:, o, :, :])
                        load_w8("wq", wq_v, o)
                    nc.sync.dma_start(bq_sb[:], bq_d[:])
                    nc.sync.dma_start(bk_sb[:], bk_d[:])
                    nc.sync.dma_start(bvb_sb[:], bvb_d[:])
                    nc.sync.dma_start(mask_sb[:], mask_d[:])
                    for o in range(KP):
                        load_w8("wk", wk_v, o)
                    # v path: fp16 x^T and wv interleaved in consumption order
                    for kc in range(KC):
                        nc.sync.dma_start(xts[kc][:], xt_v[:, kc, :])
                        load_wv(kc)
                else:
                    for o in range(KP):
                        load_w8("wq", wq_v, o)
                        load_w8("wk", wk_v, o)
                    for kc in range(KC):
                        load_wv(kc)

            wo_sb = res.tile([P, HL, C], F16, tag="wo")
            oT = res.tile([P, HL, T], F16, tag="oT")

            part_v = part_d.rearrange("(mo p) n -> p mo n", p=P)

            # ---- Phase 1 helper: projections, 8 chains interleaved across
            # all 8 PSUM banks so each arriving DMA tile is consumed by 8
            # matmuls (~1.7us of PE work per ~1.4us of DMA): the PE streams
            # at line rate during the input load.
            def p1_psums():
                tags = ["mm", "mm", "mm", "att", "att", "att", "den", "den"]
                bufs = [3, 3, 3, 3, 3, 3, 2, 2]
                return [
                    ps.tile([P, 512], F32, tag=tg, bufs=bf, name=f"p1{i}")
                    for i, (tg, bf) in enumerate(zip(tags, bufs))
                ]

            # q/k: fp8 DoubleRow, 8 chunk-pair matmuls per 512-col tile.
            # psum = SW * (x @ w); fused scale+bias moves it to SBUF fp16.
            def qk_cfg(blk, qf, kf):
                return [
                    (qf, "wq", bq_sb, rs / SW, blk * HB),
                    (kf, "wk", bk_sb, 1.0 / SW, blk * HB),
                ]

            def qk_bias(dtile, h, t, pt, bsb, sc, hoff):
                nc.vector.scalar_tensor_tensor(
                    dtile[:, h, t * 512 : (t + 1) * 512],
                    pt[:],
                    sc,
                    bsb[:, hoff + h : hoff + h + 1].to_broadcast((P, 512)),
                    ALU.mult,
                    ALU.add,
                )

            def proj_qk_wide(blk, qf, kf):
                chains = [(h, t) for h in range(HB) for t in range(T // 512)]
                for dtile, wname, bsb, sc, hoff in qk_cfg(blk, qf, kf):
                    pts = p1_psums()
                    for o in range(KP):
                        for i, (h, t) in enumerate(chains):
                            nc.tensor.matmul(
                                pts[i][:],
                                w8s[wname][o][:, :, h * DH : (h + 1) * DH],
                                xt8s[o][:, :, t * 512 : (t + 1) * 512],
                                start=(o == 0),
                                stop=(o == KP - 1),
                                perf_mode=DR,
                            )
                    for i, (h, t) in enumerate(chains):
                        qk_bias(dtile, h, t, pts[i], bsb, sc, hoff)

            def qk_units(blk, qf, kf):
                # one unit = one full q or k chain on a transient "mm" psum;
                # emitted between attention j-iterations to fill ACT-bound
                # stalls with PE work
                units = []
                for dtile, wname, bsb, sc, hoff in qk_cfg(blk, qf, kf):
                    for h in range(HB):
                        for t in range(T // 512):
                            def unit(h=h, t=t, dtile=dtile, wname=wname,
                                     bsb=bsb, sc=sc, hoff=hoff):
                                pt = ps.tile([P, 512], F32, tag="mm", name="inj")
                                for o in range(KP):
                                    nc.tensor.matmul(
                                        pt[:],
                                        w8s[wname][o][:, :, h * DH : (h + 1) * DH],
                                        xt8s[o][:, :, t * 512 : (t + 1) * 512],
                                        start=(o == 0),
                                        stop=(o == KP - 1),
                                        perf_mode=DR,
                                    )
                                qk_bias(dtile, h, t, pt, bsb, sc, hoff)
                            units.append(unit)
                return units

            def proj_v_wide(blk, vt, ms=None):
                lo = blk * BW
                ms = list(range(T // P)) if ms is None else list(ms)
                pts = p1_psums()
                for kc in range(KC):
                    for i, m in enumerate(ms):
                        nc.tensor.matmul(
                            pts[i][:],
                            xts[kc][:, m * P : (m + 1) * P],
                            wts["wv"][kc][:],
                            start=(kc == 0),
                            stop=(kc == KC - 1),
                        )
                for i, m in enumerate(ms):
                    nc.vector.tensor_tensor(
                        vt[:, m, :], pts[i][:], bvb_sb[:, lo : lo + BW], ALU.add
                    )

            def v_units(blk, vt, ms):
                lo = blk * BW
                units = []
                for m in ms:
                    def unit(m=m):
                        pt = ps.tile([P, 512], F32, tag="mm", name="injv")
                        for kc in range(KC):
                            nc.tensor.matmul(
                                pt[:],
                                xts[kc][:, m * P : (m + 1) * P],
                                wts["wv"][kc][:],
                                start=(kc == 0),
                                stop=(kc == KC - 1),
                            )
                        nc.vector.tensor_tensor(
                            vt[:, m, :], pt[:], bvb_sb[:, lo : lo + BW], ALU.add
                        )
                    units.append(unit)
                return units

            # out-projection for one (m, n2) tile; the last row's output DMA
            # is split by partition range (a [128, n] DMA always takes 128
            # per-partition descriptors ~10us on one engine; 4 parallel
            # 32-descriptor DMAs drain in ~2.5us)
            def outproj_unit(m, n2, nsplit=1):
                # nsplit > 1 splits the output DMA into column chunks issued
                # as computed: the output-DMA path sustains only ~55GB/s, so
                # the final rows must not leave a large trailing transfer
                pt = ps.tile([P, 512], F32, tag="mm", name="op")
                for h in range(HL):
                    nc.tensor.matmul(
                        pt[:],
                        oT[:, h, m * P : (m + 1) * P],
                        wo_sb[:, h, n2 * 512 : (n2 + 1) * 512],
                        start=(h == 0),
                        stop=(h == HL - 1),
                    )
                po = wp.tile([P, 512], F16, tag="po")
                cw = 512 // nsplit
                for c in range(nsplit):
                    nc.vector.tensor_copy(
                        po[:, c * cw : (c + 1) * cw], pt[:, c * cw : (c + 1) * cw]
                    )
                    nc.sync.dma_start(
                        part_v[
                            :, m, n2 * 512 + c * cw : n2 * 512 + (c + 1) * cw
                        ],
                        po[:, c * cw : (c + 1) * cw],
                    )

            def outproj_units(ms):
                return [
                    (lambda m=m, n2=n2: outproj_unit(m, n2))
                    for m in ms
                    for n2 in range(C // 512)
                ]

            class Injector:
                def __init__(self, units, start_slot, per_slot):
                    self.units = units
                    self.start = start_slot
                    self.per = per_slot
                    self.slot = 0
                    self.i = 0

                def pump(self):
                    self.slot += 1
                    if self.slot >= self.start:
                        for _ in range(min(self.per, len(self.units) - self.i)):
                            self.units[self.i]()
                            self.i += 1

                def drain(self):
                    while self.i < len(self.units):
                        self.units[self.i]()
                        self.i += 1

            # ---- Phase 2: causal attention, two heads interleaved; between
            # j-iterations the injector emits foreign PE work (next block's
            # projections / early out-proj rows) to hide the ACT exp time.
            def attention(blk, qf, kf, vt, groups, inj=None):
                for hp, qc in groups:
                    pair = (2 * hp, 2 * hp + 1)  # local head idx within block
                    jmax = (qc + 1) * 4
                    att = {}
                    den = {}
                    for l in pair:
                        att[l] = ps.tile(
                            [P, 512], F32, tag="att", bufs=3, name=f"att{l}"
                        )
                        den[l] = ps.tile(
                            [P, 512], F32, tag="den", bufs=2, name=f"den{l}"
                        )

                    def bounds(j):
                        s = max(512 * qc, 128 * j)
                        return s, 512 * qc + 512 - s

                    sts = {}

                    def issue_st(l, j):
                        s, n = bounds(j)
                        st = ps.tile([P, 512], F32, tag="mm", name=f"st{l}")
                        nc.tensor.matmul(
                            st[:, :n],
                            kf[:, l, j * P : (j + 1) * P],
                            qf[:, l, s : 512 * qc + 512],
                            start=True,
                            stop=True,
                        )
                        if 128 * j >= 512 * qc:
                            nc.vector.tensor_tensor(
                                st[:, :P], st[:, :P], mask_sb[:], ALU.add
                            )
                        sts[(l, j)] = st

                    for l in pair:
                        issue_st(l, 0)
                    for j in range(jmax):
                        s, n = bounds(j)
                        c0 = s - 512 * qc
                        for l in pair:
                            st = sts.pop((l, j))
                            E = wp.tile([P, 512], F16, tag="E", bufs=6)
                            nc.scalar.activation(E[:, :n], st[:, :n], AF.Exp)
                            if j + 1 < jmax:
                                issue_st(l, j + 1)
                            nc.tensor.matmul(
                                att[l][:, c0:],
                                vt[:, j, l * DH : (l + 1) * DH],
                                E[:, :n],
                                start=(j == 0),
                                stop=(j == jmax - 1),
                            )
                            nc.tensor.matmul(
                                den[l][:, c0:],
                                ones_sb[:],
                                E[:, :n],
                                start=(j == 0),
                                stop=(j == jmax - 1),
                            )
                        if inj is not None:
                            inj.pump()
                    for l in pair:
                        hh = blk * HB + l
                        rc = wp.tile([P, 512], F32, tag="rc")
                        nc.vector.reciprocal_approx_fast(rc[:], den[l][:])
                        nc.vector.tensor_tensor(
                            oT[:, hh, qc * 512 : (qc + 1) * 512],
                            att[l][:],
                            rc[:],
                            ALU.mult,
                        )
                if inj is not None:
                    inj.drain()

            # ---- Schedule ----
            dma_block_weights(0)
            qf0 = qkv.tile([P, HB, T], F16, tag="qf")
            kf0 = qkv.tile([P, HB, T], F16, tag="kf")
            vt0 = qkv.tile([P, T // P, BW], F16, tag="vt")
            proj_qk_wide(0, qf0, kf0)
            proj_v_wide(0, vt0)

            # blk1 weights + wo stream during blk0 attention
            dma_block_weights(1)
            nc.sync.dma_start(wo_sb[:], wo_d.rearrange("(h p) n -> p h n", p=P))

            qf1 = qkv.tile([P, HB, T], F16, tag="qf")
            kf1 = qkv.tile([P, HB, T], F16, tag="kf")
            vt1 = qkv.tile([P, T // P, BW], F16, tag="vt")

            # blk0 attention absorbs blk1's q/k projections (start at slot 5
            # so the blk1 weight DMAs have landed) plus the first half of its
            # v projection
            attention(
                0, qf0, kf0, vt0,
                groups=[(0, 0), (0, 1), (1, 0), (1, 1)],
                inj=Injector(
                    qk_units(1, qf1, kf1) + v_units(1, vt1, range(4)),
                    start_slot=5, per_slot=1,
                ),
            )
            proj_v_wide(1, vt1, ms=range(4, T // P))

            # blk1 attention runs qc-outer so oT rows m0-3 are complete after
            # the two qc=0 groups; its qc=1 groups absorb those out-proj rows
            attention(
                1, qf1, kf1, vt1,
                groups=[(0, 0), (1, 0), (0, 1), (1, 1)],
                inj=Injector(outproj_units(range(4)), start_slot=9, per_slot=1),
            )

            # ---- Phase 3: remaining out projection (rows m4-7); the last
            # two rows stream their output in column chunks so no large
            # transfer trails the final matmul ----
            for m in range(4, T // P):
                for n2 in range(C // 512):
                    outproj_unit(m, n2, nsplit=(2 if m == 6 else 4 if m == 7 else 1))

    nc.compile()
    return nc


def _prep_inputs(x, w_qkv, b_qkv, w_out):
    """Build the 8 per-core input maps (host-side shard + layout prep)."""
    f16 = np.float16
    f8 = ml_dtypes.float8_e4m3
    rs = np.float32(1.0 / np.sqrt(DH))

    BW = HB * DH

    def tile_x16(a):  # [C, T] -> [P, KC*T], chunk-major per partition
        return np.ascontiguousarray(
            a.reshape(KC, P, T).transpose(1, 0, 2).reshape(P, KC * T)
        )

    def tile_x8(a):  # [C, T] -> [P, KP*2*T], DoubleRow pair layout
        return np.ascontiguousarray(
            a.reshape(KP, 2, P, T).transpose(2, 0, 1, 3).reshape(P, KP * 2 * T)
        )

    def tile_w8(a):  # [C, 2*BW] -> [P, 2(blk)*KP*2*BW]
        return np.ascontiguousarray(
            a.reshape(KP, 2, P, 2, BW).transpose(2, 3, 0, 1, 4).reshape(P, -1)
        )

    def tile_wv(a):  # [C, 2*BW] -> [P, 2(blk)*KC*BW]
        return np.ascontiguousarray(
            a.reshape(KC, P, 2, BW).transpose(1, 2, 0, 3).reshape(P, -1)
        )

    xt = [x[b].T for b in range(B)]
    xt16 = [tile_x16(a.astype(f16)) for a in xt]
    xt8 = [tile_x8(a.astype(f8)) for a in xt]

    mask = np.where(
        np.arange(P)[None, :] >= np.arange(P)[:, None], 0.0, -1e30
    ).astype(np.float32)

    per_g = []
    for g in range(2):
        lo, hi = g * HL * DH, (g + 1) * HL * DH
        wq = tile_w8((w_qkv[:, lo:hi] * SW).astype(f8))
        wk = tile_w8((w_qkv[:, C + lo : C + hi] * SW).astype(f8))
        wv = tile_wv(w_qkv[:, 2 * C + lo : 2 * C + hi].astype(f16))
        wo = np.ascontiguousarray(w_out[lo:hi, :]).astype(f16)
        bq = (b_qkv[lo:hi] * rs).astype(np.float32).reshape(HL, P).T.copy()
        bk = b_qkv[C + lo : C + hi].astype(np.float32).reshape(HL, P).T.copy()
        bv = b_qkv[2 * C + lo : 2 * C + hi].astype(np.float32)
        bvb = np.ascontiguousarray(np.broadcast_to(bv[None, :], (P, HL * DH)))
        per_g.append(dict(wq=wq, wk=wk, wv=wv, wo=wo, bq=bq, bk=bk, bvb=bvb))

    in_maps = []
    for c in range(NCORES):
        b, g = c // 2, c % 2
        m = dict(per_g[g])
        m["xt"] = xt16[b]
        m["xt8"] = xt8[b]
        m["mask"] = mask
        in_maps.append(m)
    return in_maps


def run(x, w_qkv, b_qkv, w_out, b_out, trace=False, **trace_kwargs):
    from concourse.bass_utils import run_bass_kernel_spmd

    x = np.asarray(x, dtype=np.float32)
    w_qkv = np.asarray(w_qkv, dtype=np.float32)
    b_qkv = np.asarray(b_qkv, dtype=np.float32)
    w_out = np.asarray(w_out, dtype=np.float32)
    b_out = np.asarray(b_out, dtype=np.float32)

    if "nc" not in _cache:
        _cache["nc"] = _build()
    nc = _cache["nc"]

    in_maps = _prep_inputs(x, w_qkv, b_qkv, w_out)
    res = run_bass_kernel_spmd(
        nc, in_maps, core_ids=list(range(NCORES)), trace=trace, **trace_kwargs
    )

    out = np.empty((B, T, C), np.float32)
    for b in range(B):
        out[b] = res.results[2 * b]["part"].astype(np.float32) + res.results[
            2 * b + 1
        ]["part"].astype(np.float32)
    out += b_out
    return out, res


def kernel(x, w_qkv, b_qkv, w_out, b_out):
    out, _ = run(x, w_qkv, b_qkv, w_out, b_out)
    return out


# revision 27
# speedup vs baseline: 1.2052x; 1.2052x over previous
"""Multi-head causal self-attention (B=4, T=1024, d_model=2048, 16 heads of 128)
for 8 Trainium2 NeuronCores.

Sharding: hybrid data x tensor parallel. Core c handles batch b = c//2 and
head group g = c%2 (8 heads per core). Each core computes q/k/v projections
for its 8 heads, causal flash-style attention, and the out-projection rows
for those heads, producing a partial [1024, 2048] output for its batch.
The host sums the two partials per batch and adds the output bias.

Precision: q/k projections run in fp8(e4m3) with DoubleRow matmuls (2 k-tiles
of 128 contracted per pass -> 2x PE throughput); softmax makes the resulting
~5% q/k noise nearly invisible in the output (rel err ~1e-2 vs the 2e-2 gate)
because attention-weight wiggle only re-mixes exact fp16 v rows. The v path,
scores, AV, denominator and out-projection stay fp16 (v and out-proj errors
pass straight through to the output, so fp8 there fails the gate). Weights
are pre-scaled by 64 before fp8 quantization (w ~ +-0.022 would be subnormal
in e4m3); the 1/64 and the 1/sqrt(dh) score scale are folded into the fused
scale+bias (scalar_tensor_tensor) that moves q/k from PSUM to SBUF.

All on-device layouts are feature-major so no transposes are needed anywhere:
  - x is shipped per batch as xt [2048, 1024] twice: fp16 (v path) and fp8
    in DoubleRow pair layout (q/k path)
  - q, k are produced feature-major [dh, T] per head; v token-major [T, dh]
  - scores are computed transposed: S^T[kv, q] = k_fm.T @ q_fm (lhsT=k, rhs=q)
  - softmax denominator via ones[128,128] matmul (partition reduction on PE),
    which also broadcasts the per-q sum to all 128 partitions
  - attention output accumulates as out^T[dh, q] = v_tm.T @ exp(S^T)
  - out^T is exactly the lhsT the out-projection needs

Heads are processed in two blocks of 4 so projection weights and q/k/v
activations fit in SBUF alongside the resident x^T and w_out. Within a
block, attention is computed for two heads interleaved so PE matmuls hide
the ACT exp latency. Inputs are DMA'd in per-k-chunk tiles so the first
projection matmuls start ~1us in instead of waiting for monolithic loads.
The output partial is written fp16 (halves the tail DMA); the host sums the
two partials per batch in fp32.
"""

import numpy as np
import ml_dtypes

B, T, C = 4, 1024, 2048
H = 16          # total heads
HL = 8          # heads per core (local)
HB = 4          # heads per block
DH = 128        # head dim
KC = C // 128   # contraction chunks (16)
KP = KC // 2    # DoubleRow chunk pairs (8)
P = 128
NCORES = 8
SW = 64.0       # fp8 weight pre-scale

_cache = {}


def _build():
    import concourse.bacc as bacc
    import concourse.mybir as mybir
    import concourse.tile as tile

    F32 = mybir.dt.float32
    F16 = mybir.dt.float16
    F8 = mybir.dt.float8e4
    AF = mybir.ActivationFunctionType
    ALU = mybir.AluOpType
    DR = mybir.MatmulPerfMode.DoubleRow

    rs = float(1.0 / np.sqrt(DH))

    nc = bacc.Bacc("TRN2", target_bir_lowering=False, debug=False)

    BW = HB * DH  # head-block feature width (512)

    # all inputs ship in partition-major pre-tiled layouts so every DMA is a
    # contiguous run on both the DRAM and SBUF side. Each dma_start costs
    # ~700ns SERIALIZED on the issuing engine (DMA_DIRECT2D), so tiles are
    # batched into few large dma_starts. bq/bk/bvb/mask ship concatenated as
    # one "misc" tensor for the same reason.
    xt_d = nc.dram_tensor("xt", (P, KC * T), F16, kind="ExternalInput")
    xt8_d = nc.dram_tensor("xt8", (P, KP * 2 * T), F8, kind="ExternalInput")
    wq_d = nc.dram_tensor("wq", (P, 2 * KP * 2 * BW), F8, kind="ExternalInput")
    wk_d = nc.dram_tensor("wk", (P, 2 * KP * 2 * BW), F8, kind="ExternalInput")
    wv_d = nc.dram_tensor("wv", (P, 2 * KC * BW), F16, kind="ExternalInput")
    wo_d = nc.dram_tensor("wo", (HL * DH, C), F16, kind="ExternalInput")
    MISC = 2 * HL + HL * DH + P
    misc_d = nc.dram_tensor("misc", (P, MISC), F32, kind="ExternalInput")
    # output partial in partition-major layout [P, T/P, C]: each row DMA is a
    # 4KB-contiguous run per partition; host transposes back
    part_d = nc.dram_tensor("part", (P, (T // P) * C), F16, kind="ExternalOutput")

    XB = 4  # xt16/wv chunks per DMA batch
    xt_v = xt_d.rearrange("p (b o t) -> p b o t", b=KC // XB, o=XB)
    xt8_v = xt8_d.rearrange("p (b o two t) -> p b o two t", b=2, o=KP // 2, two=2)
    wq_v = wq_d.rearrange("p (g b o two m) -> p g b o two m", g=2, b=2, o=KP // 2, two=2)
    wk_v = wk_d.rearrange("p (g b o two m) -> p g b o two m", g=2, b=2, o=KP // 2, two=2)
    wv_v = wv_d.rearrange("p (g b o m) -> p g b o m", g=2, b=KC // XB, o=XB)

    with tile.TileContext(nc) as tc:
        with (
            tc.tile_pool(name="res", bufs=1) as res,
            tc.tile_pool(name="wblk", bufs=1) as wblk,
            tc.tile_pool(name="qkv", bufs=2) as qkv,
            tc.tile_pool(name="wp", bufs=3) as wp,
            tc.tile_pool(name="ps", bufs=3, space="PSUM") as ps,
        ):
            misc_sb = res.tile([P, MISC], F32, tag="misc")
            bq_sb = misc_sb[:, 0:HL]
            bk_sb = misc_sb[:, HL : 2 * HL]
            bvb_sb = misc_sb[:, 2 * HL : 2 * HL + HL * DH]
            mask_sb = misc_sb[:, 2 * HL + HL * DH : MISC]

            ones_sb = res.tile([P, P], F16, tag="ones")
            nc.vector.memset(ones_sb[:], 1.0)

            # Warm the PE (HAM un-throttles after ~3.4us of activity) while the
            # input DMAs stream in; these matmuls depend only on the memset.
            # The DMA path has a ~8us prolog before the first bytes land, so
            # size the warmup to roughly cover it at the ramp-up clock.
            warm = ps.tile([P, P], F32, tag="mm")
            for _ in range(80):
                nc.tensor.matmul(warm[:], ones_sb[:], ones_sb[:], start=True, stop=True)

            # x^T fp16 (v path) and fp8 DoubleRow-pair (q/k path), batched
            xts_b = [
                res.tile([P, XB, T], F16, tag=f"xt{b}", name=f"xt{b}")
                for b in range(KC // XB)
            ]
            xt8_b = [
                res.tile([P, KP // 2, 2, T], F8, tag=f"xt8_{b}", name=f"xt8_{b}")
                for b in range(2)
            ]

            def xt16(kc):
                return xts_b[kc // XB][:, kc % XB, :]

            def xt8(o):
                return xt8_b[o // (KP // 2)][:, o % (KP // 2), :, :]

            w8s = {"wq": [None, None], "wk": [None, None]}
            wvs = [None] * (KC // XB)

            def w8(wname, o):
                return w8s[wname][o // (KP // 2)][:, o % (KP // 2), :, :]

            def wv(kc):
                return wvs[kc // XB][:, kc % XB, :]

            def dma_block_weights(blk):
                def load_w8(wname, wv_, b):
                    wt = wblk.tile(
                        [P, KP // 2, 2, BW], F8, tag=f"{wname}{b}",
                        name=f"{wname}{b}_{blk}",
                    )
                    nc.sync.dma_start(wt[:], wv_[:, blk, b, :, :, :])
                    w8s[wname][b] = wt

                def load_wv(b):
                    wt = wblk.tile(
                        [P, XB, BW], F16, tag=f"wv{b}", name=f"wv{b}_{blk}"
                    )
                    nc.sync.dma_start(wt[:], wv_v[:, blk, b, :, :])
                    wvs[b] = wt

                if blk == 0:
                    for b in range(2):
                        nc.sync.dma_start(xt8_b[b][:], xt8_v[:, b, :, :, :])
                        load_w8("wq", wq_v, b)
                    nc.sync.dma_start(misc_sb[:], misc_d[:])
                    for b in range(2):
                        load_w8("wk", wk_v, b)
                    # v path: fp16 x^T and wv interleaved in consumption order
                    for b in range(KC // XB):
                        nc.sync.dma_start(xts_b[b][:], xt_v[:, b, :, :])
                        load_wv(b)
                else:
                    for b in range(2):
                        load_w8("wq", wq_v, b)
                        load_w8("wk", wk_v, b)
                    for b in range(KC // XB):
                        load_wv(b)

            wo_sb = res.tile([P, HL, C], F16, tag="wo")
            oT = res.tile([P, HL, T], F16, tag="oT")

            part_v = part_d.rearrange("p (m n) -> p m n", n=C)

            # ---- Phase 1 helper: projections, 8 chains interleaved across
            # all 8 PSUM banks so each arriving DMA tile is consumed by 8
            # matmuls (~1.7us of PE work per ~1.4us of DMA): the PE streams
            # at line rate during the input load.
            def p1_psums():
                tags = ["mm", "mm", "mm", "att", "att", "att", "den", "den"]
                bufs = [3, 3, 3, 3, 3, 3, 2, 2]
                return [
                    ps.tile([P, 512], F32, tag=tg, bufs=bf, name=f"p1{i}")
                    for i, (tg, bf) in enumerate(zip(tags, bufs))
                ]

            # q/k: fp8 DoubleRow, 8 chunk-pair matmuls per 512-col tile.
            # psum = SW * (x @ w); fused scale+bias moves it to SBUF fp16.
            def qk_cfg(blk, qf, kf):
                return [
                    (qf, "wq", bq_sb, rs / SW, blk * HB),
                    (kf, "wk", bk_sb, 1.0 / SW, blk * HB),
                ]

            def qk_bias(dtile, h, t, pt, bsb, sc, hoff):
                nc.vector.scalar_tensor_tensor(
                    dtile[:, h, t * 512 : (t + 1) * 512],
                    pt[:],
                    sc,
                    bsb[:, hoff + h : hoff + h + 1].to_broadcast((P, 512)),
                    ALU.mult,
                    ALU.add,
                )

            def proj_qk_wide(blk, qf, kf):
                chains = [(h, t) for h in range(HB) for t in range(T // 512)]
                for dtile, wname, bsb, sc, hoff in qk_cfg(blk, qf, kf):
                    pts = p1_psums()
                    for o in range(KP):
                        for i, (h, t) in enumerate(chains):
                            nc.tensor.matmul(
                                pts[i][:],
                                w8(wname, o)[:, :, h * DH : (h + 1) * DH],
                                xt8(o)[:, :, t * 512 : (t + 1) * 512],
                                start=(o == 0),
                                stop=(o == KP - 1),
                                perf_mode=DR,
                            )
                    for i, (h, t) in enumerate(chains):
                        qk_bias(dtile, h, t, pts[i], bsb, sc, hoff)

            def qk_units(blk, qf, kf):
                # one unit = one full q or k chain on a transient "mm" psum;
                # emitted between attention j-iterations to fill ACT-bound
                # stalls with PE work
                units = []
                for dtile, wname, bsb, sc, hoff in qk_cfg(blk, qf, kf):
                    for h in range(HB):
                        for t in range(T // 512):
                            def unit(h=h, t=t, dtile=dtile, wname=wname,
                                     bsb=bsb, sc=sc, hoff=hoff):
                                pt = ps.tile([P, 512], F32, tag="mm", name="inj")
                                for o in range(KP):
                                    nc.tensor.matmul(
                                        pt[:],
                                        w8(wname, o)[:, :, h * DH : (h + 1) * DH],
                                        xt8(o)[:, :, t * 512 : (t + 1) * 512],
                                        start=(o == 0),
                                        stop=(o == KP - 1),
                                        perf_mode=DR,
                                    )
                                qk_bias(dtile, h, t, pt, bsb, sc, hoff)
                            units.append(unit)
                return units

            def proj_v_wide(blk, vt, ms=None):
                lo = blk * BW
                ms = list(range(T // P)) if ms is None else list(ms)
                pts = p1_psums()
                for kc in range(KC):
                    for i, m in enumerate(ms):
                        nc.tensor.matmul(
                            pts[i][:],
                            xt16(kc)[:, m * P : (m + 1) * P],
                            wv(kc)[:],
                            start=(kc == 0),
                            stop=(kc == KC - 1),
                        )
                for i, m in enumerate(ms):
                    nc.vector.tensor_tensor(
                        vt[:, m, :], pts[i][:], bvb_sb[:, lo : lo + BW], ALU.add
                    )

            def v_units(blk, vt, ms):
                lo = blk * BW
                units = []
                for m in ms:
                    def unit(m=m):
                        pt = ps.tile([P, 512], F32, tag="mm", name="injv")
                        for kc in range(KC):
                            nc.tensor.matmul(
                                pt[:],
                                xt16(kc)[:, m * P : (m + 1) * P],
                                wv(kc)[:],
                                start=(kc == 0),
                                stop=(kc == KC - 1),
                            )
                        nc.vector.tensor_tensor(
                            vt[:, m, :], pt[:], bvb_sb[:, lo : lo + BW], ALU.add
                        )
                    units.append(unit)
                return units

            # out-projection for one (m, n2) tile. Results stage into a full
            # [P, C] row buffer; the row DMAs out once (4KB-contiguous per
            # partition in the partition-major output layout). Each dma_start
            # costs ~700ns serialized on its issuing engine, so rows go out
            # as single DMAs; the last rows split by partition range across
            # the idle Scalar/GpSimd engines so the final drain is short.
            rows = {}

            def outproj_unit(m, n2, psplit=1):
                if n2 == 0:
                    rows[m] = wp.tile([P, C], F16, tag="po", name=f"po{m}")
                row = rows[m]
                pt = ps.tile([P, 512], F32, tag="mm", name="op")
                for h in range(HL):
                    nc.tensor.matmul(
                        pt[:],
                        oT[:, h, m * P : (m + 1) * P],
                        wo_sb[:, h, n2 * 512 : (n2 + 1) * 512],
                        start=(h == 0),
                        stop=(h == HL - 1),
                    )
                nc.vector.tensor_copy(row[:, n2 * 512 : (n2 + 1) * 512], pt[:])
                if n2 == C // 512 - 1:
                    engines = [nc.sync, nc.scalar, nc.gpsimd, nc.sync]
                    pw = P // psplit
                    for c in range(psplit):
                        engines[c].dma_start(
                            part_v[c * pw : (c + 1) * pw, m, :],
                            row[c * pw : (c + 1) * pw, :],
                        )

            def outproj_units(ms):
                return [
                    (lambda m=m, n2=n2: outproj_unit(m, n2))
                    for m in ms
                    for n2 in range(C // 512)
                ]

            class Injector:
                def __init__(self, units, start_slot, per_slot):
                    self.units = units
                    self.start = start_slot
                    self.per = per_slot
                    self.slot = 0
                    self.i = 0

                def pump(self):
                    self.slot += 1
                    if self.slot >= self.start:
                        for _ in range(min(self.per, len(self.units) - self.i)):
                            self.units[self.i]()
                            self.i += 1

                def drain(self):
                    while self.i < len(self.units):
                        self.units[self.i]()
                        self.i += 1

            # ---- Phase 2: causal attention, two heads interleaved; between
            # j-iterations the injector emits foreign PE work (next block's
            # projections / early out-proj rows) to hide the ACT exp time.
            def attention(blk, qf, kf, vt, groups, inj=None):
                for hp, qc in groups:
                    pair = (2 * hp, 2 * hp + 1)  # local head idx within block
                    jmax = (qc + 1) * 4
                    att = {}
                    den = {}
                    for l in pair:
                        att[l] = ps.tile(
                            [P, 512], F32, tag="att", bufs=3, name=f"att{l}"
                        )
                        den[l] = ps.tile(
                            [P, 512], F32, tag="den", bufs=2, name=f"den{l}"
                        )

                    def bounds(j):
                        s = max(512 * qc, 128 * j)
                        return s, 512 * qc + 512 - s

                    sts = {}

                    def issue_st(l, j):
                        s, n = bounds(j)
                        st = ps.tile([P, 512], F32, tag="mm", name=f"st{l}")
                        nc.tensor.matmul(
                            st[:, :n],
                            kf[:, l, j * P : (j + 1) * P],
                            qf[:, l, s : 512 * qc + 512],
                            start=True,
                            stop=True,
                        )
                        if 128 * j >= 512 * qc:
                            nc.vector.tensor_tensor(
                                st[:, :P], st[:, :P], mask_sb[:], ALU.add
                            )
                        sts[(l, j)] = st

                    for l in pair:
                        issue_st(l, 0)
                    for j in range(jmax):
                        s, n = bounds(j)
                        c0 = s - 512 * qc
                        for l in pair:
                            st = sts.pop((l, j))
                            E = wp.tile([P, 512], F16, tag="E", bufs=6)
                            nc.scalar.activation(E[:, :n], st[:, :n], AF.Exp)
                            if j + 1 < jmax:
                                issue_st(l, j + 1)
                            nc.tensor.matmul(
                                att[l][:, c0:],
                                vt[:, j, l * DH : (l + 1) * DH],
                                E[:, :n],
                                start=(j == 0),
                                stop=(j == jmax - 1),
                            )
                            nc.tensor.matmul(
                                den[l][:, c0:],
                                ones_sb[:],
                                E[:, :n],
                                start=(j == 0),
                                stop=(j == jmax - 1),
                            )
                        if inj is not None:
                            inj.pump()
                    for l in pair:
                        hh = blk * HB + l
                        rc = wp.tile([P, 512], F32, tag="rc")
                        nc.vector.reciprocal_approx_fast(rc[:], den[l][:])
                        nc.vector.tensor_tensor(
                            oT[:, hh, qc * 512 : (qc + 1) * 512],
                            att[l][:],
                            rc[:],
                            ALU.mult,
                        )
                if inj is not None:
                    inj.drain()

            # ---- Schedule ----
            dma_block_weights(0)
            qf0 = qkv.tile([P, HB, T], F16, tag="qf")
            kf0 = qkv.tile([P, HB, T], F16, tag="kf")
            vt0 = qkv.tile([P, T // P, BW], F16, tag="vt")
            proj_qk_wide(0, qf0, kf0)
            proj_v_wide(0, vt0)

            # blk1 weights + wo stream during blk0 attention
            dma_block_weights(1)
            nc.sync.dma_start(wo_sb[:], wo_d.rearrange("(h p) n -> p h n", p=P))

            qf1 = qkv.tile([P, HB, T], F16, tag="qf")
            kf1 = qkv.tile([P, HB, T], F16, tag="kf")
            vt1 = qkv.tile([P, T // P, BW], F16, tag="vt")

            # blk0 attention absorbs blk1's q/k projections (start at slot 5
            # so the blk1 weight DMAs have landed) plus the first half of its
            # v projection
            attention(
                0, qf0, kf0, vt0,
                groups=[(0, 0), (0, 1), (1, 0), (1, 1)],
                inj=Injector(
                    qk_units(1, qf1, kf1) + v_units(1, vt1, range(4)),
                    start_slot=5, per_slot=1,
                ),
            )
            proj_v_wide(1, vt1, ms=range(4, T // P))

            # blk1 attention runs qc-outer so oT rows m0-3 are complete after
            # the two qc=0 groups; its qc=1 groups absorb those out-proj rows
            attention(
                1, qf1, kf1, vt1,
                groups=[(0, 0), (1, 0), (0, 1), (1, 1)],
                inj=Injector(outproj_units(range(4)), start_slot=9, per_slot=1),
            )

            # ---- Phase 3: remaining out projection (rows m4-7); the last
            # two rows split their output DMA across engines so no large
            # transfer trails the final matmul ----
            for m in range(4, T // P):
                for n2 in range(C // 512):
                    outproj_unit(m, n2, psplit=(2 if m == 6 else 4 if m == 7 else 1))

    nc.compile()
    return nc


def _prep_inputs(x, w_qkv, b_qkv, w_out):
    """Build the 8 per-core input maps (host-side shard + layout prep)."""
    f16 = np.float16
    f8 = ml_dtypes.float8_e4m3
    rs = np.float32(1.0 / np.sqrt(DH))

    BW = HB * DH

    def tile_x16(a):  # [C, T] -> [P, KC*T], chunk-major per partition
        return np.ascontiguousarray(
            a.reshape(KC, P, T).transpose(1, 0, 2).reshape(P, KC * T)
        )

    def tile_x8(a):  # [C, T] -> [P, KP*2*T], DoubleRow pair layout
        return np.ascontiguousarray(
            a.reshape(KP, 2, P, T).transpose(2, 0, 1, 3).reshape(P, KP * 2 * T)
        )

    def tile_w8(a):  # [C, 2*BW] -> [P, 2(blk)*KP*2*BW]
        return np.ascontiguousarray(
            a.reshape(KP, 2, P, 2, BW).transpose(2, 3, 0, 1, 4).reshape(P, -1)
        )

    def tile_wv(a):  # [C, 2*BW] -> [P, 2(blk)*KC*BW]
        return np.ascontiguousarray(
            a.reshape(KC, P, 2, BW).transpose(1, 2, 0, 3).reshape(P, -1)
        )

    xt = [x[b].T for b in range(B)]
    xt16 = [tile_x16(a.astype(f16)) for a in xt]
    xt8 = [tile_x8(a.astype(f8)) for a in xt]

    mask = np.where(
        np.arange(P)[None, :] >= np.arange(P)[:, None], 0.0, -1e30
    ).astype(np.float32)

    per_g = []
    for g in range(2):
        lo, hi = g * HL * DH, (g + 1) * HL * DH
        wq = tile_w8((w_qkv[:, lo:hi] * SW).astype(f8))
        wk = tile_w8((w_qkv[:, C + lo : C + hi] * SW).astype(f8))
        wv = tile_wv(w_qkv[:, 2 * C + lo : 2 * C + hi].astype(f16))
        wo = np.ascontiguousarray(w_out[lo:hi, :]).astype(f16)
        bq = (b_qkv[lo:hi] * rs).astype(np.float32).reshape(HL, P).T
        bk = b_qkv[C + lo : C + hi].astype(np.float32).reshape(HL, P).T
        bv = b_qkv[2 * C + lo : 2 * C + hi].astype(np.float32)
        bvb = np.broadcast_to(bv[None, :], (P, HL * DH))
        misc = np.ascontiguousarray(
            np.concatenate([bq, bk, bvb, mask], axis=1), dtype=np.float32
        )
        per_g.append(dict(wq=wq, wk=wk, wv=wv, wo=wo, misc=misc))

    in_maps = []
    for c in range(NCORES):
        b, g = c // 2, c % 2
        m = dict(per_g[g])
        m["xt"] = xt16[b]
        m["xt8"] = xt8[b]
        in_maps.append(m)
    return in_maps


def run(x, w_qkv, b_qkv, w_out, b_out, trace=False, **trace_kwargs):
    from concourse.bass_utils import run_bass_kernel_spmd

    x = np.asarray(x, dtype=np.float32)
    w_qkv = np.asarray(w_qkv, dtype=np.float32)
    b_qkv = np.asarray(b_qkv, dtype=np.float32)
    w_out = np.asarray(w_out, dtype=np.float32)
    b_out = np.asarray(b_out, dtype=np.float32)

    if "nc" not in _cache:
        _cache["nc"] = _build()
    nc = _cache["nc"]

    in_maps = _prep_inputs(x, w_qkv, b_qkv, w_out)
    res = run_bass_kernel_spmd(
        nc, in_maps, core_ids=list(range(NCORES)), trace=trace, **trace_kwargs
    )

    def unpack(arr):  # [P, (T/P)*C] partition-major -> [T, C]
        return (
            arr.reshape(P, T // P, C).transpose(1, 0, 2).reshape(T, C)
        ).astype(np.float32)

    out = np.empty((B, T, C), np.float32)
    for b in range(B):
        out[b] = unpack(res.results[2 * b]["part"]) + unpack(
            res.results[2 * b + 1]["part"]
        )
    out += b_out
    return out, res


def kernel(x, w_qkv, b_qkv, w_out, b_out):
    out, _ = run(x, w_qkv, b_qkv, w_out, b_out)
    return out


# revision 32
# speedup vs baseline: 1.2325x; 1.0226x over previous
"""Multi-head causal self-attention (B=4, T=1024, d_model=2048, 16 heads of 128)
for 8 Trainium2 NeuronCores.

Sharding: hybrid data x tensor parallel. Core c handles batch b = c//2 and
head group g = c%2 (8 heads per core). Each core computes q/k/v projections
for its 8 heads, causal flash-style attention, and the out-projection rows
for those heads, producing a partial [1024, 2048] output for its batch.
The host sums the two partials per batch and adds the output bias.

Precision: q/k projections run in fp8(e4m3) with DoubleRow matmuls (2 k-tiles
of 128 contracted per pass -> 2x PE throughput); softmax makes the resulting
~5% q/k noise nearly invisible in the output (rel err ~1e-2 vs the 2e-2 gate)
because attention-weight wiggle only re-mixes exact fp16 v rows. The v path,
scores, AV, denominator and out-projection stay fp16 (v and out-proj errors
pass straight through to the output, so fp8 there fails the gate). Weights
are pre-scaled by 64 before fp8 quantization (w ~ +-0.022 would be subnormal
in e4m3); the 1/64 and the 1/sqrt(dh) score scale are folded into the fused
scale+bias (scalar_tensor_tensor) that moves q/k from PSUM to SBUF.

All on-device layouts are feature-major so no transposes are needed anywhere:
  - x is shipped per batch as xt [2048, 1024] twice: fp16 (v path) and fp8
    in DoubleRow pair layout (q/k path)
  - q, k are produced feature-major [dh, T] per head; v token-major [T, dh]
  - scores are computed transposed: S^T[kv, q] = k_fm.T @ q_fm (lhsT=k, rhs=q)
  - softmax denominator via ones[128,128] matmul (partition reduction on PE),
    which also broadcasts the per-q sum to all 128 partitions
  - attention output accumulates as out^T[dh, q] = v_tm.T @ exp(S^T)
  - out^T is exactly the lhsT the out-projection needs

Heads are processed in two blocks of 4 so projection weights and q/k/v
activations fit in SBUF alongside the resident x^T and w_out. Within a
block, attention is computed for two heads interleaved so PE matmuls hide
the ACT exp latency. Inputs are DMA'd in per-k-chunk tiles so the first
projection matmuls start ~1us in instead of waiting for monolithic loads.
The output partial is written fp16 (halves the tail DMA); the host sums the
two partials per batch in fp32.
"""

import numpy as np
import ml_dtypes

B, T, C = 4, 1024, 2048
H = 16          # total heads
HL = 8          # heads per core (local)
HB = 4          # heads per block
DH = 128        # head dim
KC = C // 128   # contraction chunks (16)
KP = KC // 2    # DoubleRow chunk pairs (8)
P = 128
NCORES = 8
SW = 64.0       # fp8 weight pre-scale

_cache = {}


def _build():
    import concourse.bacc as bacc
    import concourse.mybir as mybir
    import concourse.tile as tile

    F32 = mybir.dt.float32
    F16 = mybir.dt.float16
    F8 = mybir.dt.float8e4
    AF = mybir.ActivationFunctionType
    ALU = mybir.AluOpType
    DR = mybir.MatmulPerfMode.DoubleRow

    rs = float(1.0 / np.sqrt(DH))

    nc = bacc.Bacc("TRN2", target_bir_lowering=False, debug=False)

    BW = HB * DH  # head-block feature width (512)

    # all inputs ship in partition-major pre-tiled layouts so every DMA is a
    # contiguous run on both the DRAM and SBUF side. Each dma_start costs
    # ~700ns SERIALIZED on the issuing engine (DMA_DIRECT2D), so tiles are
    # batched into few large dma_starts. bq/bk/bvb/mask ship concatenated as
    # one "misc" tensor for the same reason.
    xt_d = nc.dram_tensor("xt", (P, KC * T), F16, kind="ExternalInput")
    xt8_d = nc.dram_tensor("xt8", (P, KP * 2 * T), F8, kind="ExternalInput")
    wq_d = nc.dram_tensor("wq", (P, 2 * KP * 2 * BW), F8, kind="ExternalInput")
    wk_d = nc.dram_tensor("wk", (P, 2 * KP * 2 * BW), F8, kind="ExternalInput")
    wv_d = nc.dram_tensor("wv", (P, 2 * KC * BW), F16, kind="ExternalInput")
    wo_d = nc.dram_tensor("wo", (HL * DH, C), F16, kind="ExternalInput")
    MISC = 2 * HL + HL * DH + P
    misc_d = nc.dram_tensor("misc", (P, MISC), F32, kind="ExternalInput")
    # output partial in partition-major layout [P, T/P, C]: each row DMA is a
    # 4KB-contiguous run per partition; host transposes back
    part_d = nc.dram_tensor("part", (P, (T // P) * C), F16, kind="ExternalOutput")

    XB = 4  # xt16/wv chunks per DMA batch
    xt_v = xt_d.rearrange("p (b o t) -> p b o t", b=KC // XB, o=XB)
    xt8_v = xt8_d.rearrange("p (b o two t) -> p b o two t", b=2, o=KP // 2, two=2)
    wq_v = wq_d.rearrange("p (g b o two m) -> p g b o two m", g=2, b=2, o=KP // 2, two=2)
    wk_v = wk_d.rearrange("p (g b o two m) -> p g b o two m", g=2, b=2, o=KP // 2, two=2)
    wv_v = wv_d.rearrange("p (g b o m) -> p g b o m", g=2, b=KC // XB, o=XB)

    with tile.TileContext(nc) as tc:
        with (
            tc.tile_pool(name="res", bufs=1) as res,
            tc.tile_pool(name="wblk", bufs=1) as wblk,
            tc.tile_pool(name="qkv", bufs=2) as qkv,
            tc.tile_pool(name="wp", bufs=3) as wp,
            tc.tile_pool(name="ps", bufs=3, space="PSUM") as ps,
        ):
            misc_sb = res.tile([P, MISC], F32, tag="misc")
            bq_sb = misc_sb[:, 0:HL]
            bk_sb = misc_sb[:, HL : 2 * HL]
            bvb_sb = misc_sb[:, 2 * HL : 2 * HL + HL * DH]
            mask_sb = misc_sb[:, 2 * HL + HL * DH : MISC]

            ones_sb = res.tile([P, P], F16, tag="ones")
            nc.vector.memset(ones_sb[:], 1.0)

            # Warm the PE (HAM un-throttles after ~3.4us of activity) while the
            # input DMAs stream in; these matmuls depend only on the memset.
            # The DMA path has a ~8us prolog before the first bytes land, so
            # size the warmup to roughly cover it at the ramp-up clock.
            warm = ps.tile([P, P], F32, tag="mm")
            for _ in range(80):
                nc.tensor.matmul(warm[:], ones_sb[:], ones_sb[:], start=True, stop=True)

            # x^T fp16 (v path) and fp8 DoubleRow-pair (q/k path), batched
            xts_b = [
                res.tile([P, XB, T], F16, tag=f"xt{b}", name=f"xt{b}")
                for b in range(KC // XB)
            ]
            xt8_b = [
                res.tile([P, KP // 2, 2, T], F8, tag=f"xt8_{b}", name=f"xt8_{b}")
                for b in range(2)
            ]

            def xt16(kc):
                return xts_b[kc // XB][:, kc % XB, :]

            def xt8(o):
                return xt8_b[o // (KP // 2)][:, o % (KP // 2), :, :]

            w8s = {"wq": [None, None], "wk": [None, None]}
            wvs = [None] * (KC // XB)

            def w8(wname, o):
                return w8s[wname][o // (KP // 2)][:, o % (KP // 2), :, :]

            def wv(kc):
                return wvs[kc // XB][:, kc % XB, :]

            def dma_block_weights(blk):
                def load_w8(wname, wv_, b):
                    wt = wblk.tile(
                        [P, KP // 2, 2, BW], F8, tag=f"{wname}{b}",
                        name=f"{wname}{b}_{blk}",
                    )
                    nc.sync.dma_start(wt[:], wv_[:, blk, b, :, :, :])
                    w8s[wname][b] = wt

                def load_wv(b):
                    wt = wblk.tile(
                        [P, XB, BW], F16, tag=f"wv{b}", name=f"wv{b}_{blk}"
                    )
                    nc.sync.dma_start(wt[:], wv_v[:, blk, b, :, :])
                    wvs[b] = wt

                if blk == 0:
                    for b in range(2):
                        nc.sync.dma_start(xt8_b[b][:], xt8_v[:, b, :, :, :])
                        load_w8("wq", wq_v, b)
                    nc.sync.dma_start(misc_sb[:], misc_d[:])
                    for b in range(2):
                        load_w8("wk", wk_v, b)
                    # v path: fp16 x^T and wv interleaved in consumption order
                    for b in range(KC // XB):
                        nc.sync.dma_start(xts_b[b][:], xt_v[:, b, :, :])
                        load_wv(b)
                else:
                    for b in range(2):
                        load_w8("wq", wq_v, b)
                        load_w8("wk", wk_v, b)
                    for b in range(KC // XB):
                        load_wv(b)

            wo_sb = res.tile([P, HL, C], F16, tag="wo")
            oT = res.tile([P, HL, T], F16, tag="oT")

            part_v = part_d.rearrange("p (m n) -> p m n", n=C)

            # ---- Phase 1 helper: projections, 8 chains interleaved across
            # all 8 PSUM banks so each arriving DMA tile is consumed by 8
            # matmuls (~1.7us of PE work per ~1.4us of DMA): the PE streams
            # at line rate during the input load.
            def p1_psums():
                tags = ["mm", "mm", "mm", "att", "att", "att", "den", "den"]
                bufs = [3, 3, 3, 3, 3, 3, 2, 2]
                return [
                    ps.tile([P, 512], F32, tag=tg, bufs=bf, name=f"p1{i}")
                    for i, (tg, bf) in enumerate(zip(tags, bufs))
                ]

            # q/k: fp8 DoubleRow, 8 chunk-pair matmuls per 512-col tile.
            # psum = SW * (x @ w); fused scale+bias moves it to SBUF fp16.
            def qk_cfg(blk, qf, kf):
                return [
                    (qf, "wq", bq_sb, rs / SW, blk * HB),
                    (kf, "wk", bk_sb, 1.0 / SW, blk * HB),
                ]

            def qk_bias(dtile, h, t, pt, bsb, sc, hoff):
                nc.vector.scalar_tensor_tensor(
                    dtile[:, h, t * 512 : (t + 1) * 512],
                    pt[:],
                    sc,
                    bsb[:, hoff + h : hoff + h + 1].to_broadcast((P, 512)),
                    ALU.mult,
                    ALU.add,
                )

            def proj_qk_wide(blk, qf, kf):
                chains = [(h, t) for h in range(HB) for t in range(T // 512)]
                for dtile, wname, bsb, sc, hoff in qk_cfg(blk, qf, kf):
                    pts = p1_psums()
                    for o in range(KP):
                        for i, (h, t) in enumerate(chains):
                            nc.tensor.matmul(
                                pts[i][:],
                                w8(wname, o)[:, :, h * DH : (h + 1) * DH],
                                xt8(o)[:, :, t * 512 : (t + 1) * 512],
                                start=(o == 0),
                                stop=(o == KP - 1),
                                perf_mode=DR,
                            )
                    for i, (h, t) in enumerate(chains):
                        qk_bias(dtile, h, t, pts[i], bsb, sc, hoff)

            def qk_units(blk, qf, kf):
                # one unit = one full q or k chain on a transient "mm" psum;
                # emitted between attention j-iterations to fill ACT-bound
                # stalls with PE work
                units = []
                for dtile, wname, bsb, sc, hoff in qk_cfg(blk, qf, kf):
                    for h in range(HB):
                        for t in range(T // 512):
                            def unit(h=h, t=t, dtile=dtile, wname=wname,
                                     bsb=bsb, sc=sc, hoff=hoff):
                                pt = ps.tile([P, 512], F32, tag="mm", name="inj")
                                for o in range(KP):
                                    nc.tensor.matmul(
                                        pt[:],
                                        w8(wname, o)[:, :, h * DH : (h + 1) * DH],
                                        xt8(o)[:, :, t * 512 : (t + 1) * 512],
                                        start=(o == 0),
                                        stop=(o == KP - 1),
                                        perf_mode=DR,
                                    )
                                qk_bias(dtile, h, t, pt, bsb, sc, hoff)
                            units.append(unit)
                return units

            def proj_v_wide(blk, vt, ms=None):
                lo = blk * BW
                ms = list(range(T // P)) if ms is None else list(ms)
                pts = p1_psums()
                for kc in range(KC):
                    for i, m in enumerate(ms):
                        nc.tensor.matmul(
                            pts[i][:],
                            xt16(kc)[:, m * P : (m + 1) * P],
                            wv(kc)[:],
                            start=(kc == 0),
                            stop=(kc == KC - 1),
                        )
                for i, m in enumerate(ms):
                    nc.vector.tensor_tensor(
                        vt[:, m, :], pts[i][:], bvb_sb[:, lo : lo + BW], ALU.add
                    )

            def v_units(blk, vt, ms):
                lo = blk * BW
                units = []
                for m in ms:
                    def unit(m=m):
                        pt = ps.tile([P, 512], F32, tag="mm", name="injv")
                        for kc in range(KC):
                            nc.tensor.matmul(
                                pt[:],
                                xt16(kc)[:, m * P : (m + 1) * P],
                                wv(kc)[:],
                                start=(kc == 0),
                                stop=(kc == KC - 1),
                            )
                        nc.vector.tensor_tensor(
                            vt[:, m, :], pt[:], bvb_sb[:, lo : lo + BW], ALU.add
                        )
                    units.append(unit)
                return units

            # out-projection for one (m, n2) tile. Results stage into a full
            # [P, C] row buffer; the row DMAs out once (4KB-contiguous per
            # partition in the partition-major output layout). Each dma_start
            # costs ~700ns serialized on its issuing engine, so rows go out
            # as single DMAs; the last rows split by partition range across
            # the idle Scalar/GpSimd engines so the final drain is short.
            rows = {}

            def outproj_unit(m, n2, psplit=1):
                if n2 == 0:
                    rows[m] = wp.tile([P, C], F16, tag="po", name=f"po{m}")
                row = rows[m]
                pt = ps.tile([P, 512], F32, tag="mm", name="op")
                for h in range(HL):
                    nc.tensor.matmul(
                        pt[:],
                        oT[:, h, m * P : (m + 1) * P],
                        wo_sb[:, h, n2 * 512 : (n2 + 1) * 512],
                        start=(h == 0),
                        stop=(h == HL - 1),
                    )
                nc.vector.tensor_copy(row[:, n2 * 512 : (n2 + 1) * 512], pt[:])
                if n2 == C // 512 - 1:
                    engines = [nc.sync, nc.scalar, nc.gpsimd, nc.sync]
                    pw = P // psplit
                    for c in range(psplit):
                        engines[c].dma_start(
                            part_v[c * pw : (c + 1) * pw, m, :],
                            row[c * pw : (c + 1) * pw, :],
                        )

            def outproj_units(ms):
                return [
                    (lambda m=m, n2=n2: outproj_unit(m, n2))
                    for m in ms
                    for n2 in range(C // 512)
                ]

            class Injector:
                def __init__(self, units, start_slot, per_slot):
                    self.units = units
                    self.start = start_slot
                    self.per = per_slot
                    self.slot = 0
                    self.i = 0

                def pump(self):
                    self.slot += 1
                    if self.slot >= self.start:
                        for _ in range(min(self.per, len(self.units) - self.i)):
                            self.units[self.i]()
                            self.i += 1

                def drain(self):
                    while self.i < len(self.units):
                        self.units[self.i]()
                        self.i += 1

            # ---- Phase 2: causal attention, two heads interleaved; between
            # j-iterations the injector emits foreign PE work (next block's
            # projections / early out-proj rows) to hide the ACT exp time.
            # The j=0 score+exp of each group is prefetched during the
            # previous group's last iteration so the first AV matmul of a
            # group never waits on the score->mask->exp chain.
            class AttState:
                """Score/exp pipeline state for one attention pass; prologue()
                issues the first group's j=0 scores+exps and can run EARLY
                (right after the q/k projections, before the v bias-adds
                queue up on the DVE) so the first AV matmul never waits."""

                def __init__(self, qf, kf, groups):
                    self.qf, self.kf, self.groups = qf, kf, groups
                    self.sts, self.Es = {}, {}

                def g_bounds(self, gi, j):
                    qc = self.groups[gi][1]
                    s = max(512 * qc, 128 * j)
                    return s, 512 * qc + 512 - s

                def issue_st(self, gi, l, j):
                    hp, qc = self.groups[gi]
                    s, n = self.g_bounds(gi, j)
                    st = ps.tile([P, 512], F32, tag="mm", name=f"st{l}")
                    nc.tensor.matmul(
                        st[:, :n],
                        self.kf[:, l, j * P : (j + 1) * P],
                        self.qf[:, l, s : 512 * qc + 512],
                        start=True,
                        stop=True,
                    )
                    if 128 * j >= 512 * qc:
                        nc.vector.tensor_tensor(
                            st[:, :P], st[:, :P], mask_sb[:], ALU.add
                        )
                    self.sts[(gi, l, j)] = st

                def issue_exp(self, gi, l, j):
                    _, n = self.g_bounds(gi, j)
                    st = self.sts.pop((gi, l, j))
                    E = wp.tile([P, 512], F16, tag="E", bufs=6)
                    nc.scalar.activation(E[:, :n], st[:, :n], AF.Exp)
                    self.Es[(gi, l, j)] = E

                def prologue(self):
                    for l in (2 * self.groups[0][0], 2 * self.groups[0][0] + 1):
                        self.issue_st(0, l, 0)
                        self.issue_exp(0, l, 0)

            def attention(blk, vt, state, inj=None):
                groups = state.groups
                qf, kf = state.qf, state.kf
                sts, Es = state.sts, state.Es
                issue_st, issue_exp = state.issue_st, state.issue_exp
                for gi, (hp, qc) in enumerate(groups):
                    pair = (2 * hp, 2 * hp + 1)  # local head idx within block
                    jmax = (qc + 1) * 4
                    att = {}
                    den = {}
                    for l in pair:
                        att[l] = ps.tile(
                            [P, 512], F32, tag="att", bufs=3, name=f"att{l}"
                        )
                        den[l] = ps.tile(
                            [P, 512], F32, tag="den", bufs=2, name=f"den{l}"
                        )
                    for j in range(jmax):
                        s, n = state.g_bounds(gi, j)
                        c0 = s - 512 * qc
                        for l in pair:
                            if (gi, l, j) not in Es:
                                issue_exp(gi, l, j)
                            E = Es.pop((gi, l, j))
                            if j + 1 < jmax:
                                issue_st(gi, l, j + 1)
                            elif gi + 1 < len(groups):
                                # prefetch next group's j=0 score + exp
                                nhp = groups[gi + 1][0]
                                nl = 2 * nhp + (l & 1)
                                issue_st(gi + 1, nl, 0)
                                issue_exp(gi + 1, nl, 0)
                            nc.tensor.matmul(
                                att[l][:, c0:],
                                vt[:, j, l * DH : (l + 1) * DH],
                                E[:, :n],
                                start=(j == 0),
                                stop=(j == jmax - 1),
                            )
                            nc.tensor.matmul(
                                den[l][:, c0:],
                                ones_sb[:],
                                E[:, :n],
                                start=(j == 0),
                                stop=(j == jmax - 1),
                            )
                        if inj is not None:
                            inj.pump()
                    for l in pair:
                        hh = blk * HB + l
                        rc = wp.tile([P, 512], F32, tag="rc")
                        nc.vector.reciprocal_approx_fast(rc[:], den[l][:])
                        nc.vector.tensor_tensor(
                            oT[:, hh, qc * 512 : (qc + 1) * 512],
                            att[l][:],
                            rc[:],
                            ALU.mult,
                        )
                if inj is not None:
                    inj.drain()

            # ---- Schedule ----
            dma_block_weights(0)
            qf0 = qkv.tile([P, HB, T], F16, tag="qf")
            kf0 = qkv.tile([P, HB, T], F16, tag="kf")
            vt0 = qkv.tile([P, T // P, BW], F16, tag="vt")
            G0 = [(0, 0), (0, 1), (1, 0), (1, 1)]
            G1 = [(0, 0), (1, 0), (0, 1), (1, 1)]
            proj_qk_wide(0, qf0, kf0)
            st0 = AttState(qf0, kf0, G0)
            st0.prologue()
            proj_v_wide(0, vt0)

            # blk1 weights + wo stream during blk0 attention
            dma_block_weights(1)
            nc.sync.dma_start(wo_sb[:], wo_d.rearrange("(h p) n -> p h n", p=P))

            qf1 = qkv.tile([P, HB, T], F16, tag="qf")
            kf1 = qkv.tile([P, HB, T], F16, tag="kf")
            vt1 = qkv.tile([P, T // P, BW], F16, tag="vt")

            # blk0 attention absorbs blk1's q/k projections (start at slot 5
            # so the blk1 weight DMAs have landed) plus the first half of its
            # v projection
            attention(
                0, vt0, st0,
                inj=Injector(
                    qk_units(1, qf1, kf1) + v_units(1, vt1, range(4)),
                    start_slot=5, per_slot=1,
                ),
            )
            st1 = AttState(qf1, kf1, G1)
            st1.prologue()
            proj_v_wide(1, vt1, ms=range(4, T // P))

            # blk1 attention runs qc-outer so oT rows m0-3 are complete after
            # the two qc=0 groups; its qc=1 groups absorb those out-proj rows
            attention(
                1, vt1, st1,
                inj=Injector(outproj_units(range(4)), start_slot=9, per_slot=1),
            )

            # ---- Phase 3: remaining out projection (rows m4-7); the last
            # two rows split their output DMA across engines so no large
            # transfer trails the final matmul ----
            for m in range(4, T // P):
                for n2 in range(C // 512):
                    outproj_unit(m, n2, psplit=(2 if m == 6 else 4 if m == 7 else 1))

    nc.compile()
    return nc


def _prep_inputs(x, w_qkv, b_qkv, w_out):
    """Build the 8 per-core input maps (host-side shard + layout prep)."""
    f16 = np.float16
    f8 = ml_dtypes.float8_e4m3
    rs = np.float32(1.0 / np.sqrt(DH))

    BW = HB * DH

    def tile_x16(a):  # [C, T] -> [P, KC*T], chunk-major per partition
        return np.ascontiguousarray(
            a.reshape(KC, P, T).transpose(1, 0, 2).reshape(P, KC * T)
        )

    def tile_x8(a):  # [C, T] -> [P, KP*2*T], DoubleRow pair layout
        return np.ascontiguousarray(
            a.reshape(KP, 2, P, T).transpose(2, 0, 1, 3).reshape(P, KP * 2 * T)
        )

    def tile_w8(a):  # [C, 2*BW] -> [P, 2(blk)*KP*2*BW]
        return np.ascontiguousarray(
            a.reshape(KP, 2, P, 2, BW).transpose(2, 3, 0, 1, 4).reshape(P, -1)
        )

    def tile_wv(a):  # [C, 2*BW] -> [P, 2(blk)*KC*BW]
        return np.ascontiguousarray(
            a.reshape(KC, P, 2, BW).transpose(1, 2, 0, 3).reshape(P, -1)
        )

    xt = [x[b].T for b in range(B)]
    xt16 = [tile_x16(a.astype(f16)) for a in xt]
    xt8 = [tile_x8(a.astype(f8)) for a in xt]

    mask = np.where(
        np.arange(P)[None, :] >= np.arange(P)[:, None], 0.0, -1e30
    ).astype(np.float32)

    per_g = []
    for g in range(2):
        lo, hi = g * HL * DH, (g + 1) * HL * DH
        wq = tile_w8((w_qkv[:, lo:hi] * SW).astype(f8))
        wk = tile_w8((w_qkv[:, C + lo : C + hi] * SW).astype(f8))
        wv = tile_wv(w_qkv[:, 2 * C + lo : 2 * C + hi].astype(f16))
        wo = np.ascontiguousarray(w_out[lo:hi, :]).astype(f16)
        bq = (b_qkv[lo:hi] * rs).astype(np.float32).reshape(HL, P).T
        bk = b_qkv[C + lo : C + hi].astype(np.float32).reshape(HL, P).T
        bv = b_qkv[2 * C + lo : 2 * C + hi].astype(np.float32)
        bvb = np.broadcast_to(bv[None, :], (P, HL * DH))
        misc = np.ascontiguousarray(
            np.concatenate([bq, bk, bvb, mask], axis=1), dtype=np.float32
        )
        per_g.append(dict(wq=wq, wk=wk, wv=wv, wo=wo, misc=misc))

    in_maps = []
    for c in range(NCORES):
        b, g = c // 2, c % 2
        m = dict(per_g[g])
        m["xt"] = xt16[b]
        m["xt8"] = xt8[b]
        in_maps.append(m)
    return in_maps


def run(x, w_qkv, b_qkv, w_out, b_out, trace=False, **trace_kwargs):
    from concourse.bass_utils import run_bass_kernel_spmd

    x = np.asarray(x, dtype=np.float32)
    w_qkv = np.asarray(w_qkv, dtype=np.float32)
    b_qkv = np.asarray(b_qkv, dtype=np.float32)
    w_out = np.asarray(w_out, dtype=np.float32)
    b_out = np.asarray(b_out, dtype=np.float32)

    if "nc" not in _cache:
        _cache["nc"] = _build()
    nc = _cache["nc"]

    in_maps = _prep_inputs(x, w_qkv, b_qkv, w_out)
    res = run_bass_kernel_spmd(
        nc, in_maps, core_ids=list(range(NCORES)), trace=trace, **trace_kwargs
    )

    def unpack(arr):  # [P, (T/P)*C] partition-major -> [T, C]
        return (
            arr.reshape(P, T // P, C).transpose(1, 0, 2).reshape(T, C)
        ).astype(np.float32)

    out = np.empty((B, T, C), np.float32)
    for b in range(B):
        out[b] = unpack(res.results[2 * b]["part"]) + unpack(
            res.results[2 * b + 1]["part"]
        )
    out += b_out
    return out, res


def kernel(x, w_qkv, b_qkv, w_out, b_out):
    out, _ = run(x, w_qkv, b_qkv, w_out, b_out)
    return out
